# revision 8
# baseline (speedup 1.0000x reference)
"""Bass/Trainium2 kernel for nn_DreamAttention (dense transformer attention,
dead-softmax variant).

Math (per reference): q/k/v linear projections + RoPE, scores = q @ k^T /
sqrt(HD) (softmax computed but DISCARDED in the source), out = (scores @ v)
@ Wo^T.

Because no softmax is applied, attention is linear:
    (q @ k^T) @ v == q @ (k^T @ v)
so we compute the tiny per-head Gram matrix KV = k^T v  [HD, HD] instead of
the S x S score matrix.

The q-side RoPE is folded into the attention matmul (RoPE is linear):
    attn_h = KV_h^T (cos*q_h) + KVp_h^T (sin* * q_h)
where KVp is KV with its partition halves swapped and sin* carries the
rotate-half signs. The k-side RoPE is applied at PSUM-eviction time with
flat elementwise ops against host-pre-broadcast tables (a per-head
broadcast access pattern runs ~3x slower on the DVE/Pool engines than a
flat tile read, which previously made RoPE the critical path into the
collective).

Sharding: data-parallel over tokens. 8 cores x 512 tokens (cores 0-3 hold
batch 0, cores 4-7 batch 1). Each core computes q/k/v for its tokens
(weights replicated), partial per-head KV over its tokens, a bf16 AllReduce
of the KV block within each 4-core batch group (two halves, each launched
as soon as its k/v chunks exist so the collective hides under the
remaining projections), then attn and the output projection for its
tokens.

All large matmuls run in bf16 (fp32 PSUM accumulation); the PE sustains
~2 GHz of moving-operand columns and hides the stationary loads, so the
kernel is paced by total moving columns. The 1/sqrt(HD) scale is folded
into k's RoPE tables on the host.

DMA rings: sync (SP HWDGE) carries x^T + Wk + small constants + Wq + half
the y writes; scalar (ACT HWDGE) carries Wv + k-RoPE table stream + Wo +
the other y half; the gpsimd SWDGE ring carries only the collective
staging (its per-DMA software descriptor cost serializes small transfers,
so nothing latency-tolerant-but-bulky goes there).
"""

import math
from contextlib import ExitStack

import numpy as np
import ml_dtypes

import concourse.mybir as mybir
import concourse.tile as tile
from concourse import bacc
from concourse import bass_utils

P = 128
HD = 128
F32 = mybir.dt.float32
BF16 = mybir.dt.bfloat16
ADD = mybir.AluOpType.add
MULT = mybir.AluOpType.mult


def ts(i, size):
    return slice(i * size, (i + 1) * size)


def emit_attn(tc, ctx, io, t_core, d_model, replica_groups):
    """Emit the per-core attention kernel.

    io: DRAM APs: xT [d_model, t_core] bf16; wqT/wkT/wvT/woT
    [d_model, d_model] bf16; bkb/bvb [128, d_model] bf16 (broadcast
    biases); bqd [128, d_model/128] f32 (bq per-tile columns);
    coskb/sinkb [t_core, d_model] bf16 (token-major k tables,
    pre-broadcast across heads, sign-folded + 1/sqrt(HD) prescaled);
    cosqD/sinqD [128, t_core] bf16 (feature-major q tables, sinqD
    sign-folded); y [t_core, d_model] f32.
    """
    nc = tc.nc
    T_TILES = t_core // P       # 4 token tiles of 128
    DIN = d_model // P          # 16 feature tiles
    NH = d_model // HD          # 16 heads
    NCH = 512                   # psum chunk width
    CHUNKS = d_model // NCH     # 4
    TH = T_TILES // 2           # t-tiles per k/v sub-round
    HPC = NCH // HD             # heads per chunk
    h2 = HD // 2
    HS = NH // 2                # heads per collective half
    W_HALF = HS * HD

    sb = ctx.enter_context(tc.tile_pool(name="sb", bufs=1))
    ps = ctx.enter_context(tc.tile_pool(name="ps", bufs=8, space="PSUM"))
    dram = ctx.enter_context(tc.tile_pool(name="dram", bufs=4, space="DRAM"))

    def wkv_tile(name):
        return sb.tile([P, NCH], BF16, name=name, tag="wkv", bufs=44)

    def wqo_tile(name):
        return sb.tile([P, NCH], BF16, name=name, tag="wqo", bufs=32)

    def tab_tile(name):
        return sb.tile([P, NCH], BF16, name=name, tag="tab", bufs=12)

    def tmp_tile(name):
        return sb.tile([P, NCH], BF16, name=name, tag="kvtmp", bufs=8)

    def big(name):
        return sb.tile([P, d_model], BF16, name=name, tag="big", bufs=16)

    def psum(name, width=NCH):
        return ps.tile([P, width], F32, name=name, tag="ps", bufs=8)

    # ---- x^T tiles interleaved with the c0 weight streams on both
    # HWDGE rings so the first matmuls' operands land first: evens ride
    # sync with Wk, odds ride scalar with Wv ----
    xt_tiles = [sb.tile([P, t_core], BF16, name=f"xt{d}", tag="x", bufs=DIN)
                for d in range(DIN)]

    def xt(din):
        return xt_tiles[din][:]

    wk_c0, wv_c0 = [], []
    for d in range(DIN):
        if d % 2 == 0:
            nc.sync.dma_start(xt_tiles[d][:], io["xT"][ts(d, P), :])
        else:
            nc.scalar.dma_start(xt_tiles[d][:], io["xT"][ts(d, P), :])
        wt = wkv_tile(f"wk0_{d}")
        nc.sync.dma_start(wt[:], io["wkT"][ts(d, P), ts(0, NCH)])
        wk_c0.append(wt)
        wt = wkv_tile(f"wv0_{d}")
        nc.scalar.dma_start(wt[:], io["wvT"][ts(d, P), ts(0, NCH)])
        wv_c0.append(wt)

    # small constants after the c0 weights (needed ~15us in, not at t=0)
    cosq = sb.tile([P, t_core], BF16, name="cosq", tag="tabq", bufs=2)
    sinq = sb.tile([P, t_core], BF16, name="sinq", tag="tabq", bufs=2)
    nc.sync.dma_start(cosq[:], io["cosqD"][:])
    nc.sync.dma_start(sinq[:], io["sinqD"][:])
    bkb = sb.tile([P, d_model], BF16, name="bkb", tag="bias", bufs=2)
    bvb = sb.tile([P, d_model], BF16, name="bvb", tag="bias", bufs=2)
    nc.sync.dma_start(bkb[:], io["bkb"][:])
    nc.sync.dma_start(bvb[:], io["bvb"][:])
    bqd_sb = sb.tile([P, DIN], F32, name="bqd", tag="bqd", bufs=1)
    nc.gpsimd.dma_start(bqd_sb[:], io["bqd"][:])

    k_tiles = [big(f"k{t}") for t in range(T_TILES)]
    v_tiles = [big(f"v{t}") for t in range(T_TILES)]

    def kv_round(c, th):
        """One k+v projection sub-round: chunk c, token-tile half th.
        Both projections share the x^T stationary tiles; Wk rides the
        sync ring, Wv the scalar ring (tiles loaded on th==0, reused on
        th==1). The k eviction applies RoPE inline with flat ops:
          e  = psum + bias
          kc = e * cos                     (tables pre-broadcast on host)
          t2 = swap_halves(e) * sin*       (two ops, strided source)
          k~ = kc + t2
        v eviction is just psum + bias."""
        tsel = range(th * TH, (th + 1) * TH)
        kps = {t: psum(f"kp{c}_{t}") for t in tsel}
        vps = {t: psum(f"vp{c}_{t}") for t in tsel}
        for din in range(DIN):
            if c == 0 and th == 0:
                kv_round.wk[din], kv_round.wv[din] = wk_c0[din], wv_c0[din]
            elif th == 0:
                wk = wkv_tile(f"wk{c}_{din}")
                nc.sync.dma_start(wk[:], io["wkT"][ts(din, P), ts(c, NCH)])
                wv = wkv_tile(f"wv{c}_{din}")
                nc.scalar.dma_start(wv[:], io["wvT"][ts(din, P), ts(c, NCH)])
                kv_round.wk[din], kv_round.wv[din] = wk, wv
            wk, wv = kv_round.wk[din], kv_round.wv[din]
            for t in tsel:
                nc.tensor.matmul(kps[t][:], xt(din)[:, ts(t, P)],
                                 wk[:], start=(din == 0), stop=(din == DIN - 1))
                nc.tensor.matmul(vps[t][:], xt(din)[:, ts(t, P)],
                                 wv[:], start=(din == 0), stop=(din == DIN - 1))
        for t in tsel:
            ct = tab_tile(f"ckb{c}_{t}")
            nc.scalar.dma_start(ct[:], io["coskb"][ts(t, P), ts(c, NCH)])
            st = tab_tile(f"skb{c}_{t}")
            nc.scalar.dma_start(st[:], io["sinkb"][ts(t, P), ts(c, NCH)])
            # PSUM-reading evictions on DVE (Pool cannot access PSUM);
            # the SBUF-only RoPE arithmetic goes to Pool.
            ev = tmp_tile(f"e{c}_{t}")
            nc.vector.tensor_add(ev[:], kps[t][:], bkb[:, ts(c, NCH)])
            nc.vector.tensor_add(v_tiles[t][:, ts(c, NCH)], vps[t][:],
                                 bvb[:, ts(c, NCH)])
            kc = tmp_tile(f"kc{c}_{t}")
            nc.gpsimd.tensor_mul(kc[:], ev[:], ct[:])
            # strided half-swap: t2[.., 0:64] = e[.., 64:128] * sin*,
            # t2[.., 64:128] = e[.., 0:64] * sin*
            t2 = tmp_tile(f"t2{c}_{t}")
            e3 = ev[:].rearrange("p (h d) -> p h d", d=HD)
            t3 = t2[:].rearrange("p (h d) -> p h d", d=HD)
            s3 = st[:].rearrange("p (h d) -> p h d", d=HD)
            nc.gpsimd.tensor_mul(t3[:, :, 0:h2], e3[:, :, h2:HD],
                                 s3[:, :, 0:h2])
            nc.gpsimd.tensor_mul(t3[:, :, h2:HD], e3[:, :, 0:h2],
                                 s3[:, :, h2:HD])
            nc.gpsimd.tensor_add(k_tiles[t][:, ts(c, NCH)], kc[:], t2[:])
    kv_round.wk, kv_round.wv = {}, {}

    kv_sb = sb.tile([P, d_model], BF16, name="kvsb", tag="kv", bufs=3)
    kv_red = sb.tile([P, d_model], BF16, name="kvred", tag="kv", bufs=3)
    kv_perm = sb.tile([P, d_model], BF16, name="kvperm", tag="kv", bufs=3)
    cc_out = [None, None]

    def gram_half(half):
        """Partial per-head Gram KV[h] = k_h^T v_h over this core's
        tokens, for heads of one collective half, then stage + launch
        the AllReduce (bf16, 0.25 MB) on the SWDGE ring."""
        for h in range(half * HS, (half + 1) * HS):
            kvp = psum(f"kvp{h}")
            for t in range(T_TILES):
                nc.tensor.matmul(kvp[:, 0:HD], k_tiles[t][:, ts(h, HD)],
                                 v_tiles[t][:, ts(h, HD)],
                                 start=(t == 0), stop=(t == T_TILES - 1))
            nc.scalar.copy(kv_sb[:, ts(h, HD)], kvp[:, 0:HD])
        kv_in = dram.tile([P, W_HALF], BF16, name=f"kv_in{half}")
        kv_out = dram.tile([P, W_HALF], BF16, name=f"kv_out{half}")
        nc.gpsimd.dma_start(kv_in[:], kv_sb[:, ts(half, W_HALF)])
        nc.gpsimd.collective_compute(
            "AllReduce",
            mybir.AluOpType.add,
            replica_groups=replica_groups,
            ins=[kv_in.opt()],
            outs=[kv_out.opt()],
        )
        cc_out[half] = kv_out

    def fetch_reduced(half):
        """Pull the reduced KV half + its partition-half-swapped copy
        (for the folded q-side RoPE) back into SBUF."""
        kv_out = cc_out[half]
        nc.gpsimd.dma_start(kv_red[:, ts(half, W_HALF)], kv_out[:])
        nc.gpsimd.dma_start(kv_perm[0:h2, ts(half, W_HALF)], kv_out[h2:HD, :])
        nc.gpsimd.dma_start(kv_perm[h2:HD, ts(half, W_HALF)], kv_out[0:h2, :])

    qc_pack = [None] * CHUNKS
    qs_pack = [None] * CHUNKS

    def q_group(g):
        """Q projection, feature-major, one group of 4 dout tiles.
        Stationary = Wq tile, moving = x^T (all tokens). Eviction fuses
        bias add + RoPE table multiply via scalar_tensor_tensor:
        qc = (psum + bq) * cos, qs = (psum + bq) * sin*."""
        qps = [psum(f"qp{g}_{j}", t_core) for j in range(4)]
        for din in range(DIN):
            wt = wqo_tile(f"wq{g}_{din}")
            nc.sync.dma_start(wt[:], io["wqT"][ts(din, P), ts(g, 4 * P)])
            for j in range(4):
                nc.tensor.matmul(qps[j][:], wt[:, ts(j, P)], xt(din),
                                 start=(din == 0), stop=(din == DIN - 1))
        qc = big(f"qc{g}")
        qs = big(f"qs{g}")
        for j in range(4):
            dout = g * 4 + j
            nc.vector.scalar_tensor_tensor(
                qc[:, ts(j, t_core)], qps[j][:], bqd_sb[:, dout:dout + 1],
                cosq[:], ADD, MULT)
            nc.vector.scalar_tensor_tensor(
                qs[:, ts(j, t_core)], qps[j][:], bqd_sb[:, dout:dout + 1],
                sinq[:], ADD, MULT)
        qc_pack[g] = qc
        qs_pack[g] = qs

    # ---- schedule ----
    for th in range(2):
        kv_round(0, th)
    for th in range(2):
        kv_round(1, th)
    kv_round(2, 0)
    gram_half(0)            # heads 0-7: k/v chunks 0-1 are complete
    kv_round(2, 1)
    for th in range(2):
        kv_round(3, th)
    q_group(0)              # PE filler while the c3 evictions finish
    gram_half(1)
    for g in range(1, CHUNKS):
        q_group(g)
    fetch_reduced(0)
    fetch_reduced(1)

    # ---- attn_h[d2, t] = KV_h^T (cos*q_h) + KVp_h^T (sin* q_h) ----
    attn_pack = [big(f"attn{g}") for g in range(CHUNKS)]
    for h in range(NH):
        qc = qc_pack[h // 4][:, ts(h % 4, t_core)]
        qss = qs_pack[h // 4][:, ts(h % 4, t_core)]
        ap = psum(f"ap{h}", t_core)
        nc.tensor.matmul(ap[:], kv_red[:, ts(h, HD)], qc,
                         start=True, stop=False)
        nc.tensor.matmul(ap[:], kv_perm[:, ts(h, HD)], qss,
                         start=False, stop=True)
        dst = attn_pack[h // 4][:, ts(h % 4, t_core)]
        if h % 2 == 0:
            nc.scalar.copy(dst, ap[:])
        else:
            nc.vector.tensor_copy(dst, ap[:])

    # ---- Output projection: y[t, dout] = attn @ Wo^T, token-major ----
    for c in range(CHUNKS):
        ops = [psum(f"op{c}_{t}") for t in range(T_TILES)]
        for dmid in range(DIN):
            wt = wqo_tile(f"wo{c}_{dmid}")
            nc.scalar.dma_start(wt[:], io["woT"][ts(dmid, P), ts(c, NCH)])
            stat = attn_pack[dmid // 4][:, (dmid % 4) * t_core:]
            for t in range(T_TILES):
                nc.tensor.matmul(ops[t][:],
                                 stat[:, ts(t, P)], wt[:],
                                 start=(dmid == 0), stop=(dmid == DIN - 1))
        for t in range(T_TILES):
            osb = sb.tile([P, NCH], F32, name=f"osb{c}_{t}", tag="osb",
                          bufs=4)
            if (c * T_TILES + t) % 2 == 0:
                nc.vector.tensor_copy(osb[:], ops[t][:])
            else:
                nc.scalar.copy(osb[:], ops[t][:])
            eng = nc.sync if (c * T_TILES + t) % 2 == 0 else nc.scalar
            eng.dma_start(io["y"][ts(t, P), ts(c, NCH)], osb[:])


def build_nc(t_core, d_model, num_devices, replica_groups, reps=1):
    nc = bacc.Bacc("TRN2", target_bir_lowering=False, debug=False,
                   num_devices=num_devices)
    io = {}
    io["xT"] = nc.dram_tensor("xT", [d_model, t_core], BF16,
                              kind="ExternalInput").ap()
    for nm in ("wqT", "wkT", "wvT", "woT"):
        io[nm] = nc.dram_tensor(nm, [d_model, d_model], BF16,
                                kind="ExternalInput").ap()
    for nm in ("bkb", "bvb"):
        io[nm] = nc.dram_tensor(nm, [P, d_model], BF16,
                                kind="ExternalInput").ap()
    io["bqd"] = nc.dram_tensor("bqd", [P, d_model // P], F32,
                               kind="ExternalInput").ap()
    for nm in ("coskb", "sinkb"):
        io[nm] = nc.dram_tensor(nm, [t_core, d_model], BF16,
                                kind="ExternalInput").ap()
    for nm in ("cosqD", "sinqD"):
        io[nm] = nc.dram_tensor(nm, [P, t_core], BF16,
                                kind="ExternalInput").ap()
    io["y"] = nc.dram_tensor("y", [t_core, d_model], F32,
                             kind="ExternalOutput").ap()

    with tile.TileContext(nc) as tc:
        for _ in range(reps):
            with ExitStack() as ctx:
                emit_attn(tc, ctx, io, t_core, d_model, replica_groups)
    nc.compile()
    return nc


# ---------------- host side ----------------

B, S, D = 2, 2048, 2048
NH_FULL = 16
MAX_POS = 4096
ROPE_THETA = 10000.0
N_CORES = 8
T_CORE = B * S // N_CORES

_cache = {}


def _rope_tables():
    inv_freq = (np.float32(1.0) /
                np.power(np.float32(ROPE_THETA),
                         np.arange(0, HD, 2, dtype=np.float32) /
                         np.float32(HD))).astype(np.float32)
    t = np.arange(MAX_POS, dtype=np.float32)
    freqs = np.outer(t, inv_freq).astype(np.float32)
    emb = np.concatenate((freqs, freqs), axis=-1)
    return np.cos(emb).astype(np.float32), np.sin(emb).astype(np.float32)


def _get_nc():
    if "nc" not in _cache:
        _cache["nc"] = build_nc(T_CORE, D, N_CORES,
                                [[0, 1, 2, 3], [4, 5, 6, 7]])
    return _cache["nc"]


def _bf(a):
    return np.ascontiguousarray(a).astype(ml_dtypes.bfloat16)


def _host_inputs(hidden_states, position_ids, Wq, bq, Wk, bk, Wv, bv, Wo):
    x = np.asarray(hidden_states, dtype=np.float32).reshape(B * S, D)
    pos = np.asarray(position_ids).astype(np.int64).reshape(B * S)

    cos_t, sin_t = _rope_tables()
    cos = cos_t[pos]            # [B*S, HD]
    sin = sin_t[pos]
    # token-major k tables: sign-folded sin + 1/sqrt(HD) fold
    sinf = sin.copy()
    sinf[:, : HD // 2] *= np.float32(-1.0)
    scale = np.float32(1.0 / math.sqrt(HD))
    # feature-major q tables: sin* = +sin (i<64), -sin (i>=64)
    sinq = sin.copy()
    sinq[:, HD // 2:] *= np.float32(-1.0)

    wqT = _bf(np.asarray(Wq, np.float32).T)
    wkT = _bf(np.asarray(Wk, np.float32).T)
    wvT = _bf(np.asarray(Wv, np.float32).T)
    woT = _bf(np.asarray(Wo, np.float32).T)
    bkb = _bf(np.broadcast_to(np.asarray(bk, np.float32), (P, D)))
    bvb = _bf(np.broadcast_to(np.asarray(bv, np.float32), (P, D)))
    bqd = np.ascontiguousarray(
        np.asarray(bq, np.float32).reshape(D // P, P).T)

    NH = D // HD
    in_maps = []
    for c in range(N_CORES):
        sl = slice(c * T_CORE, (c + 1) * T_CORE)
        # pre-broadcast the k tables across heads: [T_CORE, D]
        coskb = _bf(np.tile(cos[sl] * scale, (1, NH)))
        sinkb = _bf(np.tile(sinf[sl] * scale, (1, NH)))
        in_maps.append({
            "xT": _bf(x[sl].T),
            "wqT": wqT, "wkT": wkT, "wvT": wvT, "woT": woT,
            "bkb": bkb, "bvb": bvb, "bqd": bqd,
            "coskb": coskb,
            "sinkb": sinkb,
            "cosqD": _bf(cos[sl].T),
            "sinqD": _bf(sinq[sl].T),
        })
    return in_maps


def kernel(hidden_states, position_ids, Wq, bq, Wk, bk, Wv, bv, Wo):
    in_maps = _host_inputs(hidden_states, position_ids,
                           Wq, bq, Wk, bk, Wv, bv, Wo)
    nc = _get_nc()
    last_err = None
    for attempt in range(3):
        try:
            res = bass_utils.run_bass_kernel_spmd(
                nc, in_maps, core_ids=list(range(N_CORES)))
            break
        except Exception as e:  # transient axon/device states clear on retry
            last_err = e
            import time
            time.sleep(15 * (attempt + 1))
    else:
        raise last_err
    out = np.concatenate([res.results[c]["y"] for c in range(N_CORES)], axis=0)
    return out.reshape(B, S, D)


# revision 10
# speedup vs baseline: 1.0772x; 1.0772x over previous
"""Bass/Trainium2 kernel for nn_DreamAttention (dense transformer attention,
dead-softmax variant).

Math (per reference): q/k/v linear projections + RoPE, scores = q @ k^T /
sqrt(HD) (softmax computed but DISCARDED in the source), out = (scores @ v)
@ Wo^T.

Because no softmax is applied, attention is linear:
    (q @ k^T) @ v == q @ (k^T @ v)
so we compute the tiny per-head Gram matrix KV = k^T v  [HD, HD] instead of
the S x S score matrix.

The q-side RoPE is folded into the attention matmul (RoPE is linear):
    attn_h = KV_h^T (cos*q_h) + KVp_h^T (sin* * q_h)
where KVp is KV with its partition halves swapped and sin* carries the
rotate-half signs. The k-side RoPE is applied at PSUM-eviction time with
flat elementwise ops against host-pre-broadcast tables (a per-head
broadcast access pattern runs ~3x slower on the DVE/Pool engines than a
flat tile read, which previously made RoPE the critical path into the
collective).

Sharding: data-parallel over tokens. 8 cores x 512 tokens (cores 0-3 hold
batch 0, cores 4-7 batch 1). Each core computes q/k/v for its tokens
(weights replicated), partial per-head KV over its tokens, a bf16 AllReduce
of the KV block within each 4-core batch group (two halves, each launched
as soon as its k/v chunks exist so the collective hides under the
remaining projections), then attn and the output projection for its
tokens.

All large matmuls run in bf16 (fp32 PSUM accumulation); the PE sustains
~2 GHz of moving-operand columns and hides the stationary loads, so the
kernel is paced by total moving columns. The 1/sqrt(HD) scale is folded
into k's RoPE tables on the host.

DMA rings: sync (SP HWDGE) carries x^T + Wk + small constants + Wq + half
the y writes; scalar (ACT HWDGE) carries Wv + k-RoPE table stream + Wo +
the other y half; the gpsimd SWDGE ring carries only the collective
staging (its per-DMA software descriptor cost serializes small transfers,
so nothing latency-tolerant-but-bulky goes there).
"""

import math
from contextlib import ExitStack

import numpy as np
import ml_dtypes

import concourse.mybir as mybir
import concourse.tile as tile
from concourse import bacc
from concourse import bass_utils

P = 128
HD = 128
F32 = mybir.dt.float32
BF16 = mybir.dt.bfloat16
ADD = mybir.AluOpType.add
MULT = mybir.AluOpType.mult


def ts(i, size):
    return slice(i * size, (i + 1) * size)


def emit_attn(tc, ctx, io, t_core, d_model, replica_groups):
    """Emit the per-core attention kernel.

    io: DRAM APs: xT [d_model, t_core] bf16; wqT/wkT/wvT/woT
    [d_model, d_model] bf16; bkb/bvb [128, d_model] bf16 (broadcast
    biases); bqd [128, d_model/128] f32 (bq per-tile columns);
    coskb/sinkb [t_core, d_model] bf16 (token-major k tables,
    pre-broadcast across heads, sign-folded + 1/sqrt(HD) prescaled);
    cosqD/sinqD [128, t_core] bf16 (feature-major q tables, sinqD
    sign-folded); y [t_core, d_model] f32.
    """
    nc = tc.nc
    T_TILES = t_core // P       # 4 token tiles of 128
    DIN = d_model // P          # 16 feature tiles
    NH = d_model // HD          # 16 heads
    NCH = 512                   # psum chunk width
    CHUNKS = d_model // NCH     # 4
    TH = T_TILES // 2           # t-tiles per k/v sub-round
    HPC = NCH // HD             # heads per chunk
    h2 = HD // 2
    HS = NH // 2                # heads per collective half
    W_HALF = HS * HD

    sb = ctx.enter_context(tc.tile_pool(name="sb", bufs=1))
    ps = ctx.enter_context(tc.tile_pool(name="ps", bufs=8, space="PSUM"))
    dram = ctx.enter_context(tc.tile_pool(name="dram", bufs=4, space="DRAM"))

    def wkv_tile(name):
        return sb.tile([P, NCH], BF16, name=name, tag="wkv", bufs=44)

    def wqo_tile(name):
        return sb.tile([P, NCH], BF16, name=name, tag="wqo", bufs=32)

    def tab_tile(name):
        return sb.tile([P, NCH], BF16, name=name, tag="tab", bufs=8)

    def tmp_tile(name):
        return sb.tile([P, NCH], BF16, name=name, tag="kvtmp", bufs=8)

    def big(name):
        return sb.tile([P, d_model], BF16, name=name, tag="big", bufs=16)

    def psum(name, width=NCH):
        return ps.tile([P, width], F32, name=name, tag="ps", bufs=8)

    # ---- x^T tiles interleaved with the c0 weight streams on both
    # HWDGE rings so the first matmuls' operands land first: evens ride
    # sync with Wk, odds ride scalar with Wv ----
    xt_tiles = [sb.tile([P, t_core], BF16, name=f"xt{d}", tag="x", bufs=DIN)
                for d in range(DIN)]

    def xt(din):
        return xt_tiles[din][:]

    wk_c0, wv_c0 = [], []
    for d in range(DIN):
        if d % 2 == 0:
            nc.sync.dma_start(xt_tiles[d][:], io["xT"][ts(d, P), :])
        else:
            nc.scalar.dma_start(xt_tiles[d][:], io["xT"][ts(d, P), :])
        wt = wkv_tile(f"wk0_{d}")
        nc.sync.dma_start(wt[:], io["wkT"][ts(d, P), ts(0, NCH)])
        wk_c0.append(wt)
        wt = wkv_tile(f"wv0_{d}")
        nc.scalar.dma_start(wt[:], io["wvT"][ts(d, P), ts(0, NCH)])
        wv_c0.append(wt)

    # small constants after the c0 weights (needed ~15us in, not at t=0)
    cosq = sb.tile([P, t_core], BF16, name="cosq", tag="tabq", bufs=2)
    sinq = sb.tile([P, t_core], BF16, name="sinq", tag="tabq", bufs=2)
    nc.sync.dma_start(cosq[:], io["cosqD"][:])
    nc.sync.dma_start(sinq[:], io["sinqD"][:])
    bkb = sb.tile([P, d_model], BF16, name="bkb", tag="bias", bufs=2)
    bvb = sb.tile([P, d_model], BF16, name="bvb", tag="bias", bufs=2)
    nc.sync.dma_start(bkb[:], io["bkb"][:])
    nc.sync.dma_start(bvb[:], io["bvb"][:])
    bqd_sb = sb.tile([P, DIN], F32, name="bqd", tag="bqd", bufs=1)
    nc.gpsimd.dma_start(bqd_sb[:], io["bqd"][:])

    k_tiles = [big(f"k{t}") for t in range(T_TILES)]
    v_tiles = [big(f"v{t}") for t in range(T_TILES)]

    # k-RoPE tables: per t-tile, one [128, NCH] tile holding the [128, HD]
    # token-major table replicated across the heads of a chunk; chunk-
    # independent, so loaded once and reused by every chunk round.
    cosk_sb, sink_sb = [], []
    for t in range(T_TILES):
        ct = tab_tile(f"ckb{t}")
        nc.scalar.dma_start(ct[:], io["coskb"][ts(t, P), :])
        st = tab_tile(f"skb{t}")
        nc.scalar.dma_start(st[:], io["sinkb"][ts(t, P), :])
        cosk_sb.append(ct)
        sink_sb.append(st)

    def kv_round(c, th):
        """One k+v projection sub-round: chunk c, token-tile half th.
        Both projections share the x^T stationary tiles; Wk rides the
        sync ring, Wv the scalar ring (tiles loaded on th==0, reused on
        th==1). The k eviction applies RoPE inline with flat ops:
          e  = psum + bias
          kc = e * cos                     (tables pre-broadcast on host)
          t2 = swap_halves(e) * sin*       (two ops, strided source)
          k~ = kc + t2
        v eviction is just psum + bias."""
        tsel = range(th * TH, (th + 1) * TH)
        kps = {t: psum(f"kp{c}_{t}") for t in tsel}
        vps = {t: psum(f"vp{c}_{t}") for t in tsel}
        for din in range(DIN):
            if c == 0 and th == 0:
                kv_round.wk[din], kv_round.wv[din] = wk_c0[din], wv_c0[din]
            elif th == 0:
                wk = wkv_tile(f"wk{c}_{din}")
                nc.sync.dma_start(wk[:], io["wkT"][ts(din, P), ts(c, NCH)])
                wv = wkv_tile(f"wv{c}_{din}")
                nc.scalar.dma_start(wv[:], io["wvT"][ts(din, P), ts(c, NCH)])
                kv_round.wk[din], kv_round.wv[din] = wk, wv
            wk, wv = kv_round.wk[din], kv_round.wv[din]
            for t in tsel:
                nc.tensor.matmul(kps[t][:], xt(din)[:, ts(t, P)],
                                 wk[:], start=(din == 0), stop=(din == DIN - 1))
                nc.tensor.matmul(vps[t][:], xt(din)[:, ts(t, P)],
                                 wv[:], start=(din == 0), stop=(din == DIN - 1))
        for t in tsel:
            ct, st = cosk_sb[t], sink_sb[t]
            # PSUM-reading evictions on DVE (Pool cannot access PSUM);
            # the SBUF-only RoPE arithmetic goes to Pool.
            ev = tmp_tile(f"e{c}_{t}")
            nc.vector.tensor_add(ev[:], kps[t][:], bkb[:, ts(c, NCH)])
            nc.vector.tensor_add(v_tiles[t][:, ts(c, NCH)], vps[t][:],
                                 bvb[:, ts(c, NCH)])
            kc = tmp_tile(f"kc{c}_{t}")
            nc.gpsimd.tensor_mul(kc[:], ev[:], ct[:])
            # strided half-swap: t2[.., 0:64] = e[.., 64:128] * sin*,
            # t2[.., 64:128] = e[.., 0:64] * sin*
            t2 = tmp_tile(f"t2{c}_{t}")
            e3 = ev[:].rearrange("p (h d) -> p h d", d=HD)
            t3 = t2[:].rearrange("p (h d) -> p h d", d=HD)
            s3 = st[:].rearrange("p (h d) -> p h d", d=HD)
            nc.gpsimd.tensor_mul(t3[:, :, 0:h2], e3[:, :, h2:HD],
                                 s3[:, :, 0:h2])
            nc.gpsimd.tensor_mul(t3[:, :, h2:HD], e3[:, :, 0:h2],
                                 s3[:, :, h2:HD])
            nc.gpsimd.tensor_add(k_tiles[t][:, ts(c, NCH)], kc[:], t2[:])
    kv_round.wk, kv_round.wv = {}, {}

    kv_sb = sb.tile([P, d_model], BF16, name="kvsb", tag="kv", bufs=3)
    kv_red = sb.tile([P, d_model], BF16, name="kvred", tag="kv", bufs=3)
    kv_perm = sb.tile([P, d_model], BF16, name="kvperm", tag="kv", bufs=3)
    cc_out = [None, None]

    def gram_half(half):
        """Partial per-head Gram KV[h] = k_h^T v_h over this core's
        tokens, for heads of one collective half, then stage + launch
        the AllReduce (bf16, 0.25 MB) on the SWDGE ring."""
        for h in range(half * HS, (half + 1) * HS):
            kvp = psum(f"kvp{h}")
            for t in range(T_TILES):
                nc.tensor.matmul(kvp[:, 0:HD], k_tiles[t][:, ts(h, HD)],
                                 v_tiles[t][:, ts(h, HD)],
                                 start=(t == 0), stop=(t == T_TILES - 1))
            nc.scalar.copy(kv_sb[:, ts(h, HD)], kvp[:, 0:HD])
        kv_in = dram.tile([P, W_HALF], BF16, name=f"kv_in{half}")
        kv_out = dram.tile([P, W_HALF], BF16, name=f"kv_out{half}")
        nc.gpsimd.dma_start(kv_in[:], kv_sb[:, ts(half, W_HALF)])
        nc.gpsimd.collective_compute(
            "AllReduce",
            mybir.AluOpType.add,
            replica_groups=replica_groups,
            ins=[kv_in.opt()],
            outs=[kv_out.opt()],
        )
        cc_out[half] = kv_out

    def fetch_reduced(half):
        """Pull the reduced KV half + its partition-half-swapped copy
        (for the folded q-side RoPE) back into SBUF."""
        kv_out = cc_out[half]
        nc.gpsimd.dma_start(kv_red[:, ts(half, W_HALF)], kv_out[:])
        nc.gpsimd.dma_start(kv_perm[0:h2, ts(half, W_HALF)], kv_out[h2:HD, :])
        nc.gpsimd.dma_start(kv_perm[h2:HD, ts(half, W_HALF)], kv_out[0:h2, :])

    qc_pack = [None] * CHUNKS
    qs_pack = [None] * CHUNKS

    def q_group(g):
        """Q projection, feature-major, one group of 4 dout tiles.
        Stationary = Wq tile, moving = x^T (all tokens). Eviction fuses
        bias add + RoPE table multiply via scalar_tensor_tensor:
        qc = (psum + bq) * cos, qs = (psum + bq) * sin*."""
        qps = [psum(f"qp{g}_{j}", t_core) for j in range(4)]
        for din in range(DIN):
            wt = wqo_tile(f"wq{g}_{din}")
            nc.sync.dma_start(wt[:], io["wqT"][ts(din, P), ts(g, 4 * P)])
            for j in range(4):
                nc.tensor.matmul(qps[j][:], wt[:, ts(j, P)], xt(din),
                                 start=(din == 0), stop=(din == DIN - 1))
        qc = big(f"qc{g}")
        qs = big(f"qs{g}")
        for j in range(4):
            dout = g * 4 + j
            nc.vector.scalar_tensor_tensor(
                qc[:, ts(j, t_core)], qps[j][:], bqd_sb[:, dout:dout + 1],
                cosq[:], ADD, MULT)
            nc.vector.scalar_tensor_tensor(
                qs[:, ts(j, t_core)], qps[j][:], bqd_sb[:, dout:dout + 1],
                sinq[:], ADD, MULT)
        qc_pack[g] = qc
        qs_pack[g] = qs

    # ---- schedule ----
    for th in range(2):
        kv_round(0, th)
    for th in range(2):
        kv_round(1, th)
    kv_round(2, 0)
    gram_half(0)            # heads 0-7: k/v chunks 0-1 are complete
    kv_round(2, 1)
    for th in range(2):
        kv_round(3, th)
    q_group(0)              # PE filler while the c3 evictions finish
    gram_half(1)
    for g in range(1, CHUNKS):
        q_group(g)
    fetch_reduced(0)
    fetch_reduced(1)

    # ---- attn_h[d2, t] = KV_h^T (cos*q_h) + KVp_h^T (sin* q_h) ----
    attn_pack = [big(f"attn{g}") for g in range(CHUNKS)]
    for h in range(NH):
        qc = qc_pack[h // 4][:, ts(h % 4, t_core)]
        qss = qs_pack[h // 4][:, ts(h % 4, t_core)]
        ap = psum(f"ap{h}", t_core)
        nc.tensor.matmul(ap[:], kv_red[:, ts(h, HD)], qc,
                         start=True, stop=False)
        nc.tensor.matmul(ap[:], kv_perm[:, ts(h, HD)], qss,
                         start=False, stop=True)
        dst = attn_pack[h // 4][:, ts(h % 4, t_core)]
        if h % 2 == 0:
            nc.scalar.copy(dst, ap[:])
        else:
            nc.vector.tensor_copy(dst, ap[:])

    # ---- Output projection: y[t, dout] = attn @ Wo^T, token-major ----
    for c in range(CHUNKS):
        ops = [psum(f"op{c}_{t}") for t in range(T_TILES)]
        for dmid in range(DIN):
            wt = wqo_tile(f"wo{c}_{dmid}")
            nc.scalar.dma_start(wt[:], io["woT"][ts(dmid, P), ts(c, NCH)])
            stat = attn_pack[dmid // 4][:, (dmid % 4) * t_core:]
            for t in range(T_TILES):
                nc.tensor.matmul(ops[t][:],
                                 stat[:, ts(t, P)], wt[:],
                                 start=(dmid == 0), stop=(dmid == DIN - 1))
        for t in range(T_TILES):
            osb = sb.tile([P, NCH], F32, name=f"osb{c}_{t}", tag="osb",
                          bufs=4)
            if (c * T_TILES + t) % 2 == 0:
                nc.vector.tensor_copy(osb[:], ops[t][:])
            else:
                nc.scalar.copy(osb[:], ops[t][:])
            eng = nc.sync if (c * T_TILES + t) % 2 == 0 else nc.scalar
            eng.dma_start(io["y"][ts(t, P), ts(c, NCH)], osb[:])


def build_nc(t_core, d_model, num_devices, replica_groups, reps=1):
    nc = bacc.Bacc("TRN2", target_bir_lowering=False, debug=False,
                   num_devices=num_devices)
    io = {}
    io["xT"] = nc.dram_tensor("xT", [d_model, t_core], BF16,
                              kind="ExternalInput").ap()
    for nm in ("wqT", "wkT", "wvT", "woT"):
        io[nm] = nc.dram_tensor(nm, [d_model, d_model], BF16,
                                kind="ExternalInput").ap()
    for nm in ("bkb", "bvb"):
        io[nm] = nc.dram_tensor(nm, [P, d_model], BF16,
                                kind="ExternalInput").ap()
    io["bqd"] = nc.dram_tensor("bqd", [P, d_model // P], F32,
                               kind="ExternalInput").ap()
    for nm in ("coskb", "sinkb"):
        io[nm] = nc.dram_tensor(nm, [t_core, 4 * HD], BF16,
                                kind="ExternalInput").ap()
    for nm in ("cosqD", "sinqD"):
        io[nm] = nc.dram_tensor(nm, [P, t_core], BF16,
                                kind="ExternalInput").ap()
    io["y"] = nc.dram_tensor("y", [t_core, d_model], F32,
                             kind="ExternalOutput").ap()

    with tile.TileContext(nc) as tc:
        for _ in range(reps):
            with ExitStack() as ctx:
                emit_attn(tc, ctx, io, t_core, d_model, replica_groups)
    nc.compile()
    return nc


# ---------------- host side ----------------

B, S, D = 2, 2048, 2048
NH_FULL = 16
MAX_POS = 4096
ROPE_THETA = 10000.0
N_CORES = 8
T_CORE = B * S // N_CORES

_cache = {}


def _rope_tables():
    inv_freq = (np.float32(1.0) /
                np.power(np.float32(ROPE_THETA),
                         np.arange(0, HD, 2, dtype=np.float32) /
                         np.float32(HD))).astype(np.float32)
    t = np.arange(MAX_POS, dtype=np.float32)
    freqs = np.outer(t, inv_freq).astype(np.float32)
    emb = np.concatenate((freqs, freqs), axis=-1)
    return np.cos(emb).astype(np.float32), np.sin(emb).astype(np.float32)


def _get_nc():
    if "nc" not in _cache:
        _cache["nc"] = build_nc(T_CORE, D, N_CORES,
                                [[0, 1, 2, 3], [4, 5, 6, 7]])
    return _cache["nc"]


def _bf(a):
    return np.ascontiguousarray(a).astype(ml_dtypes.bfloat16)


def _host_inputs(hidden_states, position_ids, Wq, bq, Wk, bk, Wv, bv, Wo):
    x = np.asarray(hidden_states, dtype=np.float32).reshape(B * S, D)
    pos = np.asarray(position_ids).astype(np.int64).reshape(B * S)

    cos_t, sin_t = _rope_tables()
    cos = cos_t[pos]            # [B*S, HD]
    sin = sin_t[pos]
    # token-major k tables: sign-folded sin + 1/sqrt(HD) fold
    sinf = sin.copy()
    sinf[:, : HD // 2] *= np.float32(-1.0)
    scale = np.float32(1.0 / math.sqrt(HD))
    # feature-major q tables: sin* = +sin (i<64), -sin (i>=64)
    sinq = sin.copy()
    sinq[:, HD // 2:] *= np.float32(-1.0)

    wqT = _bf(np.asarray(Wq, np.float32).T)
    wkT = _bf(np.asarray(Wk, np.float32).T)
    wvT = _bf(np.asarray(Wv, np.float32).T)
    woT = _bf(np.asarray(Wo, np.float32).T)
    bkb = _bf(np.broadcast_to(np.asarray(bk, np.float32), (P, D)))
    bvb = _bf(np.broadcast_to(np.asarray(bv, np.float32), (P, D)))
    bqd = np.ascontiguousarray(
        np.asarray(bq, np.float32).reshape(D // P, P).T)

    NH = D // HD
    in_maps = []
    for c in range(N_CORES):
        sl = slice(c * T_CORE, (c + 1) * T_CORE)
        # k tables replicated across the heads of one 512-chunk
        coskb = _bf(np.tile(cos[sl] * scale, (1, 4)))
        sinkb = _bf(np.tile(sinf[sl] * scale, (1, 4)))
        in_maps.append({
            "xT": _bf(x[sl].T),
            "wqT": wqT, "wkT": wkT, "wvT": wvT, "woT": woT,
            "bkb": bkb, "bvb": bvb, "bqd": bqd,
            "coskb": coskb,
            "sinkb": sinkb,
            "cosqD": _bf(cos[sl].T),
            "sinqD": _bf(sinq[sl].T),
        })
    return in_maps


def kernel(hidden_states, position_ids, Wq, bq, Wk, bk, Wv, bv, Wo):
    in_maps = _host_inputs(hidden_states, position_ids,
                           Wq, bq, Wk, bk, Wv, bv, Wo)
    nc = _get_nc()
    last_err = None
    for attempt in range(3):
        try:
            res = bass_utils.run_bass_kernel_spmd(
                nc, in_maps, core_ids=list(range(N_CORES)))
            break
        except Exception as e:  # transient axon/device states clear on retry
            last_err = e
            import time
            time.sleep(15 * (attempt + 1))
    else:
        raise last_err
    out = np.concatenate([res.results[c]["y"] for c in range(N_CORES)], axis=0)
    return out.reshape(B, S, D)
